# revision 82
# baseline (speedup 1.0000x reference)
"""Causal self-attention Trainium2 kernel (B=2, T=2048, C=1024, H=16).

Sharding: tensor-parallel over heads (4-way) x data-parallel over batch (2-way)
= 8 cores. Core c handles batch b = c//4 and heads [4*(c%4), 4*(c%4)+4).
Each core computes x @ W_attn for its head slice, causal attention for its 4
heads, and a partial y @ W_proj over its 256 channels. The host sums the 4
partials per batch element (no device collectives).

QKV runs in fp8e4m3 DoubleRow (2 k-tiles per matmul, 0.5 cyc/row):
  - x is host-split into x_hi + x_lo (error-feedback pair, both fp8).
  - W_attn columns are host-scaled by 64 (its 0.02-scale values otherwise
    land in e4m3's subnormal range) and split hi/lo.
  - q,k use QK_PRODS products (1 = x_hi*W_hi); v uses 3 products
    (x_hi*W_hi + x_lo*W_hi + x_hi*W_lo) since v-noise feeds the output
    directly. Scale compensation: exp() scale absorbs the 64^2 on scores;
    the PV denominator ones-column is 64 so y = num/den cancels v's 64.
Scores (QK^T), PV, and the projection stay fp16 (fp8 there fails the 2e-2
gate: softmax weights and v/y are too noise-sensitive).

x arrives host-pre-transposed ([C, T] contiguous), so SBUF loads are plain
DMAs - no DMA-xbar transposes gating the start.

Layouts (per core, b fixed):
  xhi/xlo [c, t] : [128, 8 ct, 2048] fp8
  qT/kT  [d', t] : per head-pair hp, [128, 2048] fp16; partitions 0-63 =
                   head 2hp, 64-127 = head 2hp+1
  vp  [s, h, d'] : [128, 16 s-tiles, 4 heads, 65] fp16; col 64 holds 64.0
                   (v's scale) so PV emits the softmax denominator for free
  sp  [s, hi, t] : scores for one s-tile, both heads of pair hp, PSUM
                   [128,2,512]; one exp covers both heads, causally trimmed
  y2  [t, h, 65] : PV output t-major in a 3-bank PSUM arena; col 64 is the
                   denominator. Normalize is a per-partition tensor_scalar
                   multiply, then a PE transpose back to [c', t] for proj.

Emission is si-outer: per s-tile, QK -> exp -> PV matmuls for every t-tile
at or above the diagonal, so the PE has PV work during the exp-bound early
s-tiles and the per-block tail is tiny. qkv matmuls for later t-blocks are
drip-fed into the stream with per-unit deadlines. Output staging copies
run on DVE (ACT for the later tiles, where its exp stream has idle);
output is fp16, upcast and summed on host.
"""

import sys

if "/opt/trn_rl_repo" not in sys.path:
    sys.path.insert(0, "/opt/trn_rl_repo")

import numpy as np
import ml_dtypes

import concourse.bass as bass
import concourse.bacc as bacc
import concourse.mybir as mybir
import concourse.tile as tile
from concourse.bass_utils import run_bass_kernel_spmd

F32 = mybir.dt.float32
F16 = mybir.dt.float16
F8 = mybir.dt.float8e4
NP_F8 = ml_dtypes.float8_e4m3
DR = mybir.MatmulPerfMode.DoubleRow

B, T, C = 2, 2048, 1024
NH = 16              # total heads
D = 64               # head dim
N_CORES = 8
HG = 4               # heads per core
FC = HG * D          # 256 f-columns per core per q/k/v
CT = C // 128        # 8 c-tiles
TT = T // 128        # 16 t-tiles / s-tiles
TB = T // 512        # 4 t-blocks
WS = 64.0            # host-side W_attn pre-scale (e4m3 subnormal dodge)
SCALE = 1.0 / (8.0 * WS * WS)   # 1/sqrt(D) / WS^2, folded into exp()
QK_PRODS = 1         # fp8 products for q,k: 1 = xhi*Whi, 2 = +xlo*Whi


def build():
    nc = bacc.Bacc("TRN2", target_bir_lowering=False, debug=False,
                   num_devices=N_CORES)
    xhi_d = nc.dram_tensor("xhi", [C, T], F8, kind="ExternalInput").ap()
    xlo_d = nc.dram_tensor("xlo", [C, T], F8, kind="ExternalInput").ap()
    wqk_d = nc.dram_tensor("wqk", [C, 2 * FC], F8, kind="ExternalInput").ap()
    wv2_d = nc.dram_tensor("wv2", [C, 2 * FC], F8, kind="ExternalInput").ap()
    wp_d = nc.dram_tensor("wp", [FC, C], F16, kind="ExternalInput").ap()
    out_d = nc.dram_tensor("out", [T, C], F16, kind="ExternalOutput").ap()

    with tile.TileContext(nc) as tc:
        body(tc, xhi_d, xlo_d, wqk_d, wv2_d, wp_d, out_d)
    nc.compile()
    return nc


def body(tc, xhi_d, xlo_d, wqk_d, wv2_d, wp_d, out_d):
    nc = tc.nc
    Exp = mybir.ActivationFunctionType.Exp

    with (
        tc.tile_pool(name="sb", bufs=1) as sb,
        tc.tile_pool(name="ps", bufs=1, space="PSUM") as ps,
    ):
        mask = sb.tile([128, 128], F16)
        ident = sb.tile([128, 128], F16)
        ones = sb.tile([128, 64], F16)
        wqk_sb = sb.tile([128, CT, 2 * FC], F8)   # [c, ct, wq|wk]
        wv2_sb = sb.tile([128, CT, 2 * FC], F8)   # [c, ct, wvh|wvl]
        wp_sb = sb.tile([128, 2, C], F16)          # [c'(128), hp, n]
        xhi = sb.tile([128, CT, T], F8)            # [c_local, ct, t]
        xlo = sb.tile([128, CT, T], F8)
        qT = sb.tile([128, 2, T], F16)             # [d', hp, t]
        kT = sb.tile([128, 2, T], F16)
        vp = sb.tile([128, TT, HG, 65], F16)       # [s_in_tile, s_tile, h, d'+1]
        pt0 = sb.tile([128, TT, 2, 512], F16)      # [s, s_tile, hi, t_in_tb]
        pt1 = sb.tile([128, TT, 2, 512], F16)
        pts = (pt0, pt1)
        yT = sb.tile([128, 2, T], F16)             # [c', hp, t]

        # Startup critical path: each HWDGE queue (SP/ACT/DVE) serializes
        # its DMAs at ~1.3us setup + transfer, so the first-QKV inputs are
        # spread across all three queues: x chunks on SP, q/k weights on
        # the (still idle) Activation queue, v weights on the DVE queue.
        # wp (first proj, ~slot 4) comes via the Pool SWDGE queue.
        for tb in range(TB):
            sl = slice(tb * 512, (tb + 1) * 512)
            nc.sync.dma_start(
                xhi[:, :, sl],
                xhi_d[:, sl].rearrange("(ct p) t -> p ct t", p=128))
            if tb == 0:
                nc.sync.dma_start(
                    wqk_sb, wqk_d.rearrange("(ct p) f -> p ct f", p=128))
                nc.sync.dma_start(
                    wv2_sb, wv2_d.rearrange("(ct p) f -> p ct f", p=128))
            nc.sync.dma_start(
                xlo[:, :, sl],
                xlo_d[:, sl].rearrange("(ct p) t -> p ct t", p=128))

        # binary causal mask in S^T orientation: 1 where t - s >= 0 else 0
        nc.gpsimd.memset(mask, 1.0)
        nc.gpsimd.affine_select(
            out=mask, in_=mask, compare_op=mybir.AluOpType.is_ge,
            fill=0.0, base=0, pattern=[[1, 128]], channel_multiplier=-1)
        # identity for PE transposes: intersect the two triangle selects
        nc.gpsimd.memset(ident, 1.0)
        nc.gpsimd.affine_select(
            out=ident, in_=ident, compare_op=mybir.AluOpType.is_ge,
            fill=0.0, base=0, pattern=[[1, 128]], channel_multiplier=-1)
        nc.gpsimd.affine_select(
            out=ident, in_=ident, compare_op=mybir.AluOpType.is_ge,
            fill=0.0, base=0, pattern=[[-1, 128]], channel_multiplier=1)
        nc.gpsimd.memset(ones, 1.0)
        nc.gpsimd.memset(vp[:, :, :, 64:65], WS)   # PV denominator column
        nc.gpsimd.dma_start(wp_sb, wp_d.rearrange("(hp p) n -> p hp n", p=128))

        # PSUM: sp 2x2 banks + qv 1 + arena 3 = 8 banks
        def sp_tile(name):
            return ps.tile([128, 2, 512], F32, tag="sp", name=name, bufs=2)

        def qv_tile(name):
            return ps.tile([128, 512], F32, tag="qv", name=name, bufs=1)

        # 3-bank arena: 16 PV accumulation regions of 65 fp32 columns
        # (region r = ttl*4+h at bank r//7, slot r%7). PSUM start_tensor_calc
        # zeroes a whole 2KB bank, which would wipe sibling regions, so the
        # arena is zeroed by DVE memset per t-block and every PV matmul
        # accumulates with start=False.
        arena = ps.tile([128, 1536], F32, name="arena")

        def reg(h, ttl):
            r = ttl * 4 + h
            bank, idx = divmod(r, 7)
            c0 = bank * 512 + idx * 65
            return arena[:, c0:c0 + 65]

        # HAM warmup: the PE is DMA-blocked at startup; dependency-free
        # matmuls on constants un-throttle the clock, and one tiny exp
        # prefetches the spline table.
        for _ in range(6):
            warm = qv_tile("warm")
            nc.tensor.matmul(warm[0:64, 0:128], lhsT=ones, rhs=mask,
                             start=True, stop=True)
            nc.tensor.matmul(warm[0:64, 128:256], lhsT=ones, rhs=mask,
                             start=True, stop=True)
        warm_e = sb.tile([1, 64], F16)
        nc.scalar.activation(warm_e, ones[0:1, :], Exp, scale=SCALE)

        def emit_v(tt):
            # 3-product fp8 DoubleRow chain: xhi*Wh + xlo*Wh + xhi*Wl.
            # Output lives in the arena's bank-2 tail (not qv), so v chains
            # don't serialize with the q/k GEMM chains through one bank.
            # start=True would zero PV regions 14/15 in the same bank, so
            # the region is memset (Pool) and the chain runs start=False.
            v_ps = qv_tile("v_ps")[:, 0:256]
            prods = ((xhi, 0), (xlo, 0), (xhi, FC))
            n = len(prods) * 4
            i = 0
            for xp, f0 in prods:
                for j in range(4):
                    nc.tensor.matmul(
                        v_ps,
                        lhsT=xp[:, 2 * j:2 * j + 2,
                                tt * 128:(tt + 1) * 128],
                        rhs=wv2_sb[:, 2 * j:2 * j + 2, f0:f0 + FC],
                        start=(i == 0), stop=(i == n - 1), perf_mode=DR)
                    i += 1
            if tt >= 12:
                # these pop in the tb2->tb3 boundary window where the DVE
                # is the bottleneck and the exp stream has a hole
                nc.scalar.copy(
                    vp[:, tt, :, 0:64],
                    v_ps.rearrange("p (h d) -> p h d", h=HG))
            else:
                nc.vector.tensor_copy(
                    vp[:, tt, :, 0:64],
                    v_ps.rearrange("p (h d) -> p h d", h=HG))

        def emit_qk(hp, f0, dst, tb):
            qk_ps = qv_tile("qk_ps")
            xparts = (xhi, xlo)[:QK_PRODS]
            n = len(xparts) * 4
            i = 0
            for xp in xparts:
                for j in range(4):
                    nc.tensor.matmul(
                        qk_ps,
                        lhsT=wqk_sb[:, 2 * j:2 * j + 2,
                                    f0 + hp * 128:f0 + (hp + 1) * 128],
                        rhs=xp[:, 2 * j:2 * j + 2,
                               tb * 512:(tb + 1) * 512],
                        start=(i == 0), stop=(i == n - 1), perf_mode=DR)
                    i += 1
            nc.vector.tensor_copy(
                dst[:, hp, tb * 512:(tb + 1) * 512], qk_ps)

        def emit_qk_si(hp, tb, si):
            # scores S^T for one s-tile, both heads of pair hp, then one
            # exp covering both heads with the causal prefix trimmed
            kd = si - 4 * tb
            col0 = 128 * kd if kd > 0 else 0
            sp = sp_tile("sp")
            for hi in (0, 1):
                nc.tensor.matmul(
                    sp[:, hi, col0:512],
                    lhsT=kT[64 * hi:64 * hi + 64, hp,
                            si * 128:(si + 1) * 128],
                    rhs=qT[64 * hi:64 * hi + 64, hp,
                           tb * 512 + col0:(tb + 1) * 512],
                    start=True, stop=True)
            pt = pts[hp]
            nc.scalar.activation(pt[:, si, :, col0:512], sp[:, :, col0:512],
                                 Exp, scale=SCALE)
            if kd >= 0:
                # zero the invalid triangle of the diagonal square after
                # exp (exp * 0 == masked exp, off the S -> exp hot path);
                # on Pool - SBUF-only op, and the Pool queue is quiet
                for hi in (0, 1):
                    psl = pt[:, si, hi, col0:col0 + 128]
                    nc.gpsimd.tensor_mul(psl, psl, mask)

        def emit_pv_si(tb, si):
            # PV matmuls of s-tile si into every t-tile at/above the
            # diagonal; chain (h, ttl) accumulates over si and closes at
            # the diagonal
            for ttl in range(max(0, si - 4 * tb), 4):
                tt = 4 * tb + ttl
                for h in range(HG):
                    hp, hi = h // 2, h % 2
                    nc.tensor.matmul(
                        reg(h, ttl),
                        lhsT=pts[hp][:, si, hi, ttl * 128:(ttl + 1) * 128],
                        rhs=vp[:, si, h, :],
                        start=False, stop=(si == tt),
                        skip_group_check=True)

        def region_runs(ttl):
            """maximal same-bank runs of the 4 regions of t-tile ttl"""
            runs = []
            h0 = 0
            while h0 < HG:
                r0 = ttl * 4 + h0
                n = min(HG - h0, 7 - r0 % 7)
                bank, idx = divmod(r0, 7)
                runs.append((h0, n, bank * 512 + idx * 65))
                h0 += n
            return runs

        # normalize is split into two backlog units so the PE transpose is
        # emitted a slot after the DVE multiply that feeds it (the in-order
        # PE queue would otherwise stall mid-chain waiting on the DVE)
        def emit_norm_a(tb, tt):
            ttl = tt - 4 * tb
            runs = region_runs(ttl)
            rcp = sb.tile([128, HG], F32, tag="rcp", name="rcp", bufs=3)
            for h0, n, base in runs:
                sl = arena[:, base:base + 65 * n].rearrange(
                    "p (n x) -> p n x", n=n)
                nc.vector.reciprocal(rcp[:, h0:h0 + n], sl[:, :, 64])
            y_sb = sb.tile([128, FC], F16, tag="ysb", name="y_sb", bufs=3)
            for h0, n, base in runs:
                sl = arena[:, base:base + 65 * n].rearrange(
                    "p (n x) -> p n x", n=n)
                nc.vector.tensor_mul(
                    y_sb[:, h0 * 64:(h0 + n) * 64].rearrange(
                        "p (n x) -> p n x", n=n),
                    sl[:, :, 0:64],
                    rcp[:, h0:h0 + n].unsqueeze(2).broadcast_to([128, n, 64]))
            return y_sb

        def emit_norm_b(tb, tt, y_sb):
            ttl = tt - 4 * tb
            # PE transpose [t, c'] -> [c', t] for the projection lhsT; the
            # scratch borrows an sp rotation slot (start=True zeroing is
            # safe there, unlike in the accumulating arena)
            if tt >= 12:
                # post-exp-stream: idle sp bank, off the qv serial chain
                yt = sp_tile("yt")[:, 0, 0:128].bitcast(F16)
            else:
                yt = qv_tile("yt")[:, 0:128].bitcast(F16)  # [128, 256] f16
            for ch in range(2):
                nc.tensor.transpose(
                    yt[:, ch * 128:(ch + 1) * 128],
                    y_sb[:, ch * 128:(ch + 1) * 128], ident)
            nc.vector.tensor_copy(
                yT[:, :, tt * 128:(tt + 1) * 128],
                yt.rearrange("p (hp t) -> p hp t", hp=2))
            # re-zero this t-tile's regions for the next block's start=False
            # accumulation chains (WAR-ordered after the reads above); the
            # last block has no successor - skipping it shortens the tail
            if tb + 1 < TB:
                for h0, n, base in region_runs(ttl):
                    nc.vector.memset(arena[:, base:base + 65 * n], 0.0)

        def emit_proj(tt):
            # two 1-bank qv passes instead of a 2-bank sp borrow: the sp
            # rotation then serves only the QK+exp stream. The last tiles
            # run after the exp stream ends, so they use the idle sp banks
            # (parallel, not qv-serialized) to shorten the tail.
            if tt >= 12:
                pjt = sp_tile("pj")
                pj = [pjt[:, 0, :], pjt[:, 1, :]]
            else:
                pj = [qv_tile("pj0"), qv_tile("pj1")]
            for nb in range(2):
                for hp in range(2):
                    nc.tensor.matmul(
                        pj[nb],
                        lhsT=yT[:, hp, tt * 128:(tt + 1) * 128],
                        rhs=wp_sb[:, hp, nb * 512:(nb + 1) * 512],
                        start=(hp == 0), stop=(hp == 1))
            # GPSIMD cannot access PSUM (BIR verifier): DVE evacuates, and
            # the Activation engine (idle once its exp stream is done)
            # takes over for the final tiles to shorten the tail
            ob = sb.tile([128, C], F16, tag="ob", name="ob", bufs=3)
            if tt >= 8:
                nc.scalar.copy(ob[:, 0:512], pj[0])
                nc.scalar.copy(ob[:, 512:1024], pj[1])
            else:
                nc.vector.tensor_copy(ob[:, 0:512], pj[0])
                nc.vector.tensor_copy(ob[:, 512:1024], pj[1])
            nc.sync.dma_start(out_d[tt * 128:(tt + 1) * 128, :], ob)

        # ---- global exp-stream emission ----
        # The Activation engine (exp) is the roofline; emission follows one
        # global (tb, si) stream so its QK+exp pairs are never queued behind
        # bulk PE work. All other PE/DVE/Pool work (qkv GEMMs, PV, norm,
        # proj) is a FIFO backlog of (ready, deadline, cycles, fn) units
        # drained between stream slots: a unit is held until its `ready`
        # slot (so cross-engine producers from slot i have a full slot of
        # wall time before an in-order consumer is emitted), forced at its
        # `deadline`, and otherwise paced to even PE-cycle rate. FIFO order
        # is load-bearing for pv->norm->proj chains and norm-before-next-
        # block-PV (arena reuse); Tile semaphores enforce the data deps.
        stream = [(tb, si) for tb in range(TB) for si in range(4 * tb + 4)]
        idx = {p: i for i, p in enumerate(stream)}
        nslots = len(stream)

        # initial zeroing of the PV regions (start=False chains)
        for ttl in range(4):
            for h0, n, base in region_runs(ttl):
                nc.vector.memset(arena[:, base:base + 65 * n], 0.0)

        # tb0 prologue: only head-pair 0's q/k GEMMs before the stream; the
        # hp1 GEMMs are emitted inside slot 0 between the hp0 and hp1
        # QK+exp pairs, so the first exp isn't queued behind them
        emit_qk(0, 0, qT, 0)
        emit_qk(0, FC, kT, 0)

        backlog = []   # (ready, deadline, pe_cycles, fn) FIFO
        staged = []    # (push_slot, ready, deadline, pe_cycles, fn)
        QKG = 256 * 4 * QK_PRODS        # emit_qk PE cycles
        VG = 128 * 12                   # emit_v PE cycles

        for tt in range(4):
            backlog.append((0, tt + 2, VG, lambda tt=tt: emit_v(tt)))

        def push_block_prefetch(tb):
            """qk GEMMs + v tiles of block tb, pushed during block tb-1."""
            i0 = idx[(tb, 0)]
            for j, (hp, f0, dst) in enumerate(
                    ((0, 0, qT), (0, FC, kT), (1, 0, qT), (1, FC, kT))):
                backlog.append((0, i0 - 3 + (j + 1) // 2, QKG,
                                lambda hp=hp, f0=f0, d=dst, tb=tb:
                                emit_qk(hp, f0, d, tb)))
            for ttl in range(4):
                tt = 4 * tb + ttl
                backlog.append((0, idx[(tb, min(tt, 4 * tb + 3))] - 1, VG,
                                lambda tt=tt: emit_v(tt)))

        total_filler = 4 * VG
        for tb in range(1, TB):
            total_filler += 4 * QKG + 4 * VG
        for tb, si in stream:
            ttl0 = max(0, si - 4 * tb)
            total_filler += (4 - ttl0) * 4 * 65          # PV
            if si >= 4 * tb:
                total_filler += 2 * 128 + 4 * 512        # norm transposes+proj

        # per-slot filler capacity = exp wall-time at 2.4GHz minus the
        # slot's mandatory QK cycles; pacing follows cumulative capacity so
        # loaded early blocks shed filler into the roomy late blocks
        cap = []
        for i, (tb, si) in enumerate(stream):
            cols = 512 - (128 * (si - 4 * tb) if si >= 4 * tb else 0)
            act_cyc = 2 * (2 * cols + 444)               # 2 exps, 1.2GHz*2
            c = max(0.0, act_cyc * 2.0 - 4 * cols)
            if i < 8:
                # startup DMA and warm-up eat the early slots' wall time
                c *= 0.5
            cap.append(c)
        cum = 0.0
        cap_cum = []
        for c in cap:
            cum += c
            cap_cum.append(cum)

        spent = 0
        for i, (tb, si) in enumerate(stream):
            # staged units whose push slot arrived enter the FIFO first
            # (before this slot's own pushes) - ordering matters for the
            # norm_b-before-next-block-PV arena constraint
            for u in [u for u in staged if u[0] <= i]:
                staged.remove(u)
                backlog.append(u[1:])
            if si == 0 and tb + 1 < TB:
                push_block_prefetch(tb + 1)
            for hp in range(2):
                if (tb, si, hp) == (0, 0, 1):
                    emit_qk(1, 0, qT, 0)
                    emit_qk(1, FC, kT, 0)
                emit_qk_si(hp, tb, si)
            # units for this slot's PV (and norm/proj at diagonal slots);
            # in the last block the chains drain inline (no next block to
            # hide them in), so their deadlines are tight
            last = tb + 1 >= TB
            if not last:
                d_pv = idx[(tb + 1, si)] - 1
            else:
                d_pv = min(i + 1, nslots - 1)
            ttl0 = max(0, si - 4 * tb)
            backlog.append((i + 1, d_pv, (4 - ttl0) * 4 * 65,
                            lambda tb=tb, si=si: emit_pv_si(tb, si)))
            if si >= 4 * tb:
                tt = si
                cell = {}

                def norm_a(tb=tb, tt=tt, cell=cell):
                    cell["y"] = emit_norm_a(tb, tt)

                def norm_b(tb=tb, tt=tt, cell=cell):
                    emit_norm_b(tb, tt, cell["y"])

                # non-last blocks: FIFO order (not deadlines) guarantees
                # norm-before-next-block-PV, so norm/proj are purely paced
                d_n = min(i + 2, nslots - 1) if last else nslots - 1
                d_p = min(i + 3, nslots - 1) if last else nslots - 1
                staged.append((i + 1, min(i + 1, nslots - 1), d_n, 0,
                               norm_a))
                staged.append((i + 1, min(i + 2, nslots - 1), d_n, 2 * 128,
                               norm_b))
                staged.append((i + 2, min(i + 3, nslots - 1), d_p, 4 * 512,
                               lambda tt=tt: emit_proj(tt)))
            # drain: everything up to the deepest due unit, then pace ready
            # units to the capacity-weighted rate
            budget = total_filler * cap_cum[i] / cap_cum[-1]
            due = max((j for j, u in enumerate(backlog) if u[1] <= i),
                      default=-1)
            while backlog and (due >= 0 or
                               (backlog[0][0] <= i and spent < budget)):
                _, _, cyc, fn = backlog.pop(0)
                due -= 1
                fn()
                spent += cyc
        for u in staged:
            backlog.append(u[1:])
        for _, _, _, fn in backlog:
            fn()


_NC_CACHE = None


def _get_nc():
    global _NC_CACHE
    if _NC_CACHE is None:
        _NC_CACHE = build()
    return _NC_CACHE


def _hilo8(a):
    hi = a.astype(NP_F8)
    lo = (a - hi.astype(np.float32)).astype(NP_F8)
    return hi, lo


def _in_maps(x, W_attn, W_proj):
    wp16 = W_proj.astype(np.float16)
    was = W_attn * WS
    maps = []
    for core in range(N_CORES):
        b, g = core // 4, core % 4
        f0 = FC * g
        xT = np.ascontiguousarray(x[b].T)           # [C, T] fp32
        xh, xl = _hilo8(xT)
        wq8 = np.ascontiguousarray(was[:, f0:f0 + FC]).astype(NP_F8)
        wk8 = np.ascontiguousarray(
            was[:, C + f0:C + f0 + FC]).astype(NP_F8)
        wv = np.ascontiguousarray(was[:, 2 * C + f0:2 * C + f0 + FC])
        wvh, wvl = _hilo8(wv)
        maps.append({
            "xhi": xh,
            "xlo": xl,
            "wqk": np.ascontiguousarray(np.concatenate([wq8, wk8], axis=1)),
            "wv2": np.ascontiguousarray(np.concatenate([wvh, wvl], axis=1)),
            "wp": np.ascontiguousarray(wp16[f0:f0 + FC, :]),
        })
    return maps


def run(x, W_attn, W_proj, trace=False, **kwargs):
    nc = _get_nc()
    res = run_bass_kernel_spmd(nc, _in_maps(x, W_attn, W_proj),
                               core_ids=list(range(N_CORES)),
                               trace=trace, **kwargs)
    out = np.zeros((B, T, C), dtype=np.float32)
    for core in range(N_CORES):
        out[core // 4] += res.results[core]["out"].astype(np.float32)
    return out, res


def kernel(x, W_attn, W_proj):
    x = np.asarray(x, dtype=np.float32)
    W_attn = np.asarray(W_attn, dtype=np.float32)
    W_proj = np.asarray(W_proj, dtype=np.float32)
    out, _ = run(x, W_attn, W_proj, trace=False)
    return out


# revision 83
# speedup vs baseline: 1.0031x; 1.0031x over previous
"""Causal self-attention Trainium2 kernel (B=2, T=2048, C=1024, H=16).

Sharding: tensor-parallel over heads (4-way) x data-parallel over batch (2-way)
= 8 cores. Core c handles batch b = c//4 and heads [4*(c%4), 4*(c%4)+4).
Each core computes x @ W_attn for its head slice, causal attention for its 4
heads, and a partial y @ W_proj over its 256 channels. The host sums the 4
partials per batch element (no device collectives).

QKV runs in fp8e4m3 DoubleRow (2 k-tiles per matmul, 0.5 cyc/row):
  - x is host-split into x_hi + x_lo (error-feedback pair, both fp8).
  - W_attn columns are host-scaled by 64 (its 0.02-scale values otherwise
    land in e4m3's subnormal range) and split hi/lo.
  - q,k use QK_PRODS products (1 = x_hi*W_hi); v uses 3 products
    (x_hi*W_hi + x_lo*W_hi + x_hi*W_lo) since v-noise feeds the output
    directly. Scale compensation: exp() scale absorbs the 64^2 on scores;
    the PV denominator ones-column is 64 so y = num/den cancels v's 64.
Scores (QK^T), PV, and the projection stay fp16 (fp8 there fails the 2e-2
gate: softmax weights and v/y are too noise-sensitive).

x arrives host-pre-transposed ([C, T] contiguous), so SBUF loads are plain
DMAs - no DMA-xbar transposes gating the start.

Layouts (per core, b fixed):
  xhi/xlo [c, t] : [128, 8 ct, 2048] fp8
  qT/kT  [d', t] : per head-pair hp, [128, 2048] fp16; partitions 0-63 =
                   head 2hp, 64-127 = head 2hp+1
  vp  [s, h, d'] : [128, 16 s-tiles, 4 heads, 65] fp16; col 64 holds 64.0
                   (v's scale) so PV emits the softmax denominator for free
  sp  [s, hi, t] : scores for one s-tile, both heads of pair hp, PSUM
                   [128,2,512]; one exp covers both heads, causally trimmed
  y2  [t, h, 65] : PV output t-major in a 3-bank PSUM arena; col 64 is the
                   denominator. Normalize is a per-partition tensor_scalar
                   multiply, then a PE transpose back to [c', t] for proj.

Emission is si-outer: per s-tile, QK -> exp -> PV matmuls for every t-tile
at or above the diagonal, so the PE has PV work during the exp-bound early
s-tiles and the per-block tail is tiny. qkv matmuls for later t-blocks are
drip-fed into the stream with per-unit deadlines. Output staging copies
run on DVE (ACT for the later tiles, where its exp stream has idle);
output is fp16, upcast and summed on host.
"""

import sys

if "/opt/trn_rl_repo" not in sys.path:
    sys.path.insert(0, "/opt/trn_rl_repo")

import numpy as np
import ml_dtypes

import concourse.bass as bass
import concourse.bacc as bacc
import concourse.mybir as mybir
import concourse.tile as tile
from concourse.bass_utils import run_bass_kernel_spmd

F32 = mybir.dt.float32
F16 = mybir.dt.float16
F8 = mybir.dt.float8e4
NP_F8 = ml_dtypes.float8_e4m3
DR = mybir.MatmulPerfMode.DoubleRow

B, T, C = 2, 2048, 1024
NH = 16              # total heads
D = 64               # head dim
N_CORES = 8
HG = 4               # heads per core
FC = HG * D          # 256 f-columns per core per q/k/v
CT = C // 128        # 8 c-tiles
TT = T // 128        # 16 t-tiles / s-tiles
TB = T // 512        # 4 t-blocks
WS = 64.0            # host-side W_attn pre-scale (e4m3 subnormal dodge)
SCALE = 1.0 / (8.0 * WS * WS)   # 1/sqrt(D) / WS^2, folded into exp()
QK_PRODS = 1         # fp8 products for q,k: 1 = xhi*Whi, 2 = +xlo*Whi


def build():
    nc = bacc.Bacc("TRN2", target_bir_lowering=False, debug=False,
                   num_devices=N_CORES)
    xhi_d = nc.dram_tensor("xhi", [C, T], F8, kind="ExternalInput").ap()
    xlo_d = nc.dram_tensor("xlo", [C, T], F8, kind="ExternalInput").ap()
    wqk_d = nc.dram_tensor("wqk", [C, 2 * FC], F8, kind="ExternalInput").ap()
    wv2_d = nc.dram_tensor("wv2", [C, 2 * FC], F8, kind="ExternalInput").ap()
    wp_d = nc.dram_tensor("wp", [FC, C], F16, kind="ExternalInput").ap()
    out_d = nc.dram_tensor("out", [T, C], F16, kind="ExternalOutput").ap()

    with tile.TileContext(nc) as tc:
        body(tc, xhi_d, xlo_d, wqk_d, wv2_d, wp_d, out_d)
    nc.compile()
    return nc


def body(tc, xhi_d, xlo_d, wqk_d, wv2_d, wp_d, out_d):
    nc = tc.nc
    Exp = mybir.ActivationFunctionType.Exp

    with (
        tc.tile_pool(name="sb", bufs=1) as sb,
        tc.tile_pool(name="ps", bufs=1, space="PSUM") as ps,
    ):
        mask = sb.tile([128, 128], F16)
        ident = sb.tile([128, 128], F16)
        ones = sb.tile([128, 64], F16)
        wqk_sb = sb.tile([128, CT, 2 * FC], F8)   # [c, ct, wq|wk]
        wv2_sb = sb.tile([128, CT, 2 * FC], F8)   # [c, ct, wvh|wvl]
        wp_sb = sb.tile([128, 2, C], F16)          # [c'(128), hp, n]
        xhi = sb.tile([128, CT, T], F8)            # [c_local, ct, t]
        xlo = sb.tile([128, CT, T], F8)
        qT = sb.tile([128, 2, T], F16)             # [d', hp, t]
        kT = sb.tile([128, 2, T], F16)
        vp = sb.tile([128, TT, HG, 65], F16)       # [s_in_tile, s_tile, h, d'+1]
        pt0 = sb.tile([128, TT, 2, 512], F16)      # [s, s_tile, hi, t_in_tb]
        pt1 = sb.tile([128, TT, 2, 512], F16)
        pts = (pt0, pt1)
        yT = sb.tile([128, 2, T], F16)             # [c', hp, t]

        # Startup critical path: each HWDGE queue (SP/ACT/DVE) serializes
        # its DMAs at ~1.3us setup + transfer, so the first-QKV inputs are
        # spread across all three queues: x chunks on SP, q/k weights on
        # the (still idle) Activation queue, v weights on the DVE queue.
        # wp (first proj, ~slot 4) comes via the Pool SWDGE queue.
        for tb in range(TB):
            sl = slice(tb * 512, (tb + 1) * 512)
            nc.sync.dma_start(
                xhi[:, :, sl],
                xhi_d[:, sl].rearrange("(ct p) t -> p ct t", p=128))
            if tb == 0:
                nc.sync.dma_start(
                    wqk_sb, wqk_d.rearrange("(ct p) f -> p ct f", p=128))
                nc.sync.dma_start(
                    wv2_sb, wv2_d.rearrange("(ct p) f -> p ct f", p=128))
            nc.sync.dma_start(
                xlo[:, :, sl],
                xlo_d[:, sl].rearrange("(ct p) t -> p ct t", p=128))

        # binary causal mask in S^T orientation: 1 where t - s >= 0 else 0
        nc.gpsimd.memset(mask, 1.0)
        nc.gpsimd.affine_select(
            out=mask, in_=mask, compare_op=mybir.AluOpType.is_ge,
            fill=0.0, base=0, pattern=[[1, 128]], channel_multiplier=-1)
        # identity for PE transposes: intersect the two triangle selects
        nc.gpsimd.memset(ident, 1.0)
        nc.gpsimd.affine_select(
            out=ident, in_=ident, compare_op=mybir.AluOpType.is_ge,
            fill=0.0, base=0, pattern=[[1, 128]], channel_multiplier=-1)
        nc.gpsimd.affine_select(
            out=ident, in_=ident, compare_op=mybir.AluOpType.is_ge,
            fill=0.0, base=0, pattern=[[-1, 128]], channel_multiplier=1)
        nc.gpsimd.memset(ones, 1.0)
        nc.gpsimd.memset(vp[:, :, :, 64:65], WS)   # PV denominator column
        nc.gpsimd.dma_start(wp_sb, wp_d.rearrange("(hp p) n -> p hp n", p=128))

        # PSUM: sp 2x2 banks + qv 1 + arena 3 = 8 banks
        def sp_tile(name):
            return ps.tile([128, 2, 512], F32, tag="sp", name=name, bufs=2)

        def qv_tile(name):
            return ps.tile([128, 512], F32, tag="qv", name=name, bufs=1)

        # 3-bank arena: 16 PV accumulation regions of 65 fp32 columns
        # (region r = ttl*4+h at bank r//7, slot r%7). PSUM start_tensor_calc
        # zeroes a whole 2KB bank, which would wipe sibling regions, so the
        # arena is zeroed by DVE memset per t-block and every PV matmul
        # accumulates with start=False.
        arena = ps.tile([128, 1536], F32, name="arena")

        def reg(h, ttl):
            r = ttl * 4 + h
            bank, idx = divmod(r, 7)
            c0 = bank * 512 + idx * 65
            return arena[:, c0:c0 + 65]

        # HAM warmup: the PE is DMA-blocked at startup; dependency-free
        # matmuls on constants un-throttle the clock, and one tiny exp
        # prefetches the spline table.
        for _ in range(6):
            warm = qv_tile("warm")
            nc.tensor.matmul(warm[0:64, 0:128], lhsT=ones, rhs=mask,
                             start=True, stop=True)
            nc.tensor.matmul(warm[0:64, 128:256], lhsT=ones, rhs=mask,
                             start=True, stop=True)
        warm_e = sb.tile([1, 64], F16)
        nc.scalar.activation(warm_e, ones[0:1, :], Exp, scale=SCALE)

        def emit_v(tt):
            # 3-product fp8 DoubleRow chain: xhi*Wh + xlo*Wh + xhi*Wl.
            # Output lives in the arena's bank-2 tail (not qv), so v chains
            # don't serialize with the q/k GEMM chains through one bank.
            # start=True would zero PV regions 14/15 in the same bank, so
            # the region is memset (Pool) and the chain runs start=False.
            v_ps = qv_tile("v_ps")[:, 0:256]
            prods = ((xhi, 0), (xlo, 0), (xhi, FC))
            n = len(prods) * 4
            i = 0
            for xp, f0 in prods:
                for j in range(4):
                    nc.tensor.matmul(
                        v_ps,
                        lhsT=xp[:, 2 * j:2 * j + 2,
                                tt * 128:(tt + 1) * 128],
                        rhs=wv2_sb[:, 2 * j:2 * j + 2, f0:f0 + FC],
                        start=(i == 0), stop=(i == n - 1), perf_mode=DR)
                    i += 1
            if tt >= 12:
                # these pop in the tb2->tb3 boundary window where the DVE
                # is the bottleneck and the exp stream has a hole
                nc.scalar.copy(
                    vp[:, tt, :, 0:64],
                    v_ps.rearrange("p (h d) -> p h d", h=HG))
            else:
                nc.vector.tensor_copy(
                    vp[:, tt, :, 0:64],
                    v_ps.rearrange("p (h d) -> p h d", h=HG))

        def emit_qk(hp, f0, dst, tb):
            qk_ps = qv_tile("qk_ps")
            xparts = (xhi, xlo)[:QK_PRODS]
            n = len(xparts) * 4
            i = 0
            for xp in xparts:
                for j in range(4):
                    nc.tensor.matmul(
                        qk_ps,
                        lhsT=wqk_sb[:, 2 * j:2 * j + 2,
                                    f0 + hp * 128:f0 + (hp + 1) * 128],
                        rhs=xp[:, 2 * j:2 * j + 2,
                               tb * 512:(tb + 1) * 512],
                        start=(i == 0), stop=(i == n - 1), perf_mode=DR)
                    i += 1
            nc.vector.tensor_copy(
                dst[:, hp, tb * 512:(tb + 1) * 512], qk_ps)

        def emit_qk_si(hp, tb, si):
            # scores S^T for one s-tile, both heads of pair hp, then one
            # exp covering both heads with the causal prefix trimmed
            kd = si - 4 * tb
            col0 = 128 * kd if kd > 0 else 0
            sp = sp_tile("sp")
            for hi in (0, 1):
                nc.tensor.matmul(
                    sp[:, hi, col0:512],
                    lhsT=kT[64 * hi:64 * hi + 64, hp,
                            si * 128:(si + 1) * 128],
                    rhs=qT[64 * hi:64 * hi + 64, hp,
                           tb * 512 + col0:(tb + 1) * 512],
                    start=True, stop=True)
            pt = pts[hp]
            nc.scalar.activation(pt[:, si, :, col0:512], sp[:, :, col0:512],
                                 Exp, scale=SCALE)
            if kd >= 0:
                # zero the invalid triangle of the diagonal square after
                # exp (exp * 0 == masked exp, off the S -> exp hot path);
                # on Pool - SBUF-only op, and the Pool queue is quiet
                for hi in (0, 1):
                    psl = pt[:, si, hi, col0:col0 + 128]
                    nc.gpsimd.tensor_mul(psl, psl, mask)

        def emit_pv_si(tb, si):
            # PV matmuls of s-tile si into every t-tile at/above the
            # diagonal; chain (h, ttl) accumulates over si and closes at
            # the diagonal
            for ttl in range(max(0, si - 4 * tb), 4):
                tt = 4 * tb + ttl
                for h in range(HG):
                    hp, hi = h // 2, h % 2
                    nc.tensor.matmul(
                        reg(h, ttl),
                        lhsT=pts[hp][:, si, hi, ttl * 128:(ttl + 1) * 128],
                        rhs=vp[:, si, h, :],
                        start=False, stop=(si == tt),
                        skip_group_check=True)

        def region_runs(ttl):
            """maximal same-bank runs of the 4 regions of t-tile ttl"""
            runs = []
            h0 = 0
            while h0 < HG:
                r0 = ttl * 4 + h0
                n = min(HG - h0, 7 - r0 % 7)
                bank, idx = divmod(r0, 7)
                runs.append((h0, n, bank * 512 + idx * 65))
                h0 += n
            return runs

        # normalize is split into two backlog units so the PE transpose is
        # emitted a slot after the DVE multiply that feeds it (the in-order
        # PE queue would otherwise stall mid-chain waiting on the DVE)
        def emit_norm_a(tb, tt):
            ttl = tt - 4 * tb
            runs = region_runs(ttl)
            rcp = sb.tile([128, HG], F32, tag="rcp", name="rcp", bufs=3)
            for h0, n, base in runs:
                sl = arena[:, base:base + 65 * n].rearrange(
                    "p (n x) -> p n x", n=n)
                nc.vector.reciprocal(rcp[:, h0:h0 + n], sl[:, :, 64])
            y_sb = sb.tile([128, FC], F16, tag="ysb", name="y_sb", bufs=3)
            for h0, n, base in runs:
                sl = arena[:, base:base + 65 * n].rearrange(
                    "p (n x) -> p n x", n=n)
                nc.vector.tensor_mul(
                    y_sb[:, h0 * 64:(h0 + n) * 64].rearrange(
                        "p (n x) -> p n x", n=n),
                    sl[:, :, 0:64],
                    rcp[:, h0:h0 + n].unsqueeze(2).broadcast_to([128, n, 64]))
            return y_sb

        def emit_norm_b(tb, tt, y_sb):
            ttl = tt - 4 * tb
            # PE transpose [t, c'] -> [c', t] for the projection lhsT; the
            # scratch borrows an sp rotation slot (start=True zeroing is
            # safe there, unlike in the accumulating arena)
            yt = qv_tile("yt")[:, 0:128].bitcast(F16)      # [128, 256] f16
            for ch in range(2):
                nc.tensor.transpose(
                    yt[:, ch * 128:(ch + 1) * 128],
                    y_sb[:, ch * 128:(ch + 1) * 128], ident)
            nc.vector.tensor_copy(
                yT[:, :, tt * 128:(tt + 1) * 128],
                yt.rearrange("p (hp t) -> p hp t", hp=2))
            # re-zero this t-tile's regions for the next block's start=False
            # accumulation chains (WAR-ordered after the reads above); the
            # last block has no successor - skipping it shortens the tail
            if tb + 1 < TB:
                for h0, n, base in region_runs(ttl):
                    nc.vector.memset(arena[:, base:base + 65 * n], 0.0)

        def emit_proj(tt):
            # two 1-bank qv passes instead of a 2-bank sp borrow: the sp
            # rotation then serves only the QK+exp stream. The last tiles
            # run after the exp stream ends, so they use the idle sp banks
            # (parallel, not qv-serialized) to shorten the tail.
            if tt >= 12:
                pjt = sp_tile("pj")
                pj = [pjt[:, 0, :], pjt[:, 1, :]]
            else:
                pj = [qv_tile("pj0"), qv_tile("pj1")]
            for nb in range(2):
                for hp in range(2):
                    nc.tensor.matmul(
                        pj[nb],
                        lhsT=yT[:, hp, tt * 128:(tt + 1) * 128],
                        rhs=wp_sb[:, hp, nb * 512:(nb + 1) * 512],
                        start=(hp == 0), stop=(hp == 1))
            # GPSIMD cannot access PSUM (BIR verifier): DVE evacuates, and
            # the Activation engine (idle once its exp stream is done)
            # takes over for the final tiles to shorten the tail
            ob = sb.tile([128, C], F16, tag="ob", name="ob", bufs=3)
            if tt >= 8:
                nc.scalar.copy(ob[:, 0:512], pj[0])
                nc.scalar.copy(ob[:, 512:1024], pj[1])
            else:
                nc.vector.tensor_copy(ob[:, 0:512], pj[0])
                nc.vector.tensor_copy(ob[:, 512:1024], pj[1])
            nc.sync.dma_start(out_d[tt * 128:(tt + 1) * 128, :], ob)

        # ---- global exp-stream emission ----
        # The Activation engine (exp) is the roofline; emission follows one
        # global (tb, si) stream so its QK+exp pairs are never queued behind
        # bulk PE work. All other PE/DVE/Pool work (qkv GEMMs, PV, norm,
        # proj) is a FIFO backlog of (ready, deadline, cycles, fn) units
        # drained between stream slots: a unit is held until its `ready`
        # slot (so cross-engine producers from slot i have a full slot of
        # wall time before an in-order consumer is emitted), forced at its
        # `deadline`, and otherwise paced to even PE-cycle rate. FIFO order
        # is load-bearing for pv->norm->proj chains and norm-before-next-
        # block-PV (arena reuse); Tile semaphores enforce the data deps.
        stream = [(tb, si) for tb in range(TB) for si in range(4 * tb + 4)]
        idx = {p: i for i, p in enumerate(stream)}
        nslots = len(stream)

        # initial zeroing of the PV regions (start=False chains)
        for ttl in range(4):
            for h0, n, base in region_runs(ttl):
                nc.vector.memset(arena[:, base:base + 65 * n], 0.0)

        # tb0 prologue: only head-pair 0's q/k GEMMs before the stream; the
        # hp1 GEMMs are emitted inside slot 0 between the hp0 and hp1
        # QK+exp pairs, so the first exp isn't queued behind them
        emit_qk(0, 0, qT, 0)
        emit_qk(0, FC, kT, 0)

        backlog = []   # (ready, deadline, pe_cycles, fn) FIFO
        staged = []    # (push_slot, ready, deadline, pe_cycles, fn)
        QKG = 256 * 4 * QK_PRODS        # emit_qk PE cycles
        VG = 128 * 12                   # emit_v PE cycles

        for tt in range(4):
            backlog.append((0, tt + 2, VG, lambda tt=tt: emit_v(tt)))

        def push_block_prefetch(tb):
            """qk GEMMs + v tiles of block tb, pushed during block tb-1."""
            i0 = idx[(tb, 0)]
            for j, (hp, f0, dst) in enumerate(
                    ((0, 0, qT), (0, FC, kT), (1, 0, qT), (1, FC, kT))):
                backlog.append((0, i0 - 3 + (j + 1) // 2, QKG,
                                lambda hp=hp, f0=f0, d=dst, tb=tb:
                                emit_qk(hp, f0, d, tb)))
            for ttl in range(4):
                tt = 4 * tb + ttl
                backlog.append((0, idx[(tb, min(tt, 4 * tb + 3))] - 1, VG,
                                lambda tt=tt: emit_v(tt)))

        total_filler = 4 * VG
        for tb in range(1, TB):
            total_filler += 4 * QKG + 4 * VG
        for tb, si in stream:
            ttl0 = max(0, si - 4 * tb)
            total_filler += (4 - ttl0) * 4 * 65          # PV
            if si >= 4 * tb:
                total_filler += 2 * 128 + 4 * 512        # norm transposes+proj

        # per-slot filler capacity = exp wall-time at 2.4GHz minus the
        # slot's mandatory QK cycles; pacing follows cumulative capacity so
        # loaded early blocks shed filler into the roomy late blocks
        cap = []
        for i, (tb, si) in enumerate(stream):
            cols = 512 - (128 * (si - 4 * tb) if si >= 4 * tb else 0)
            act_cyc = 2 * (2 * cols + 444)               # 2 exps, 1.2GHz*2
            c = max(0.0, act_cyc * 2.0 - 4 * cols)
            if i < 8:
                # startup DMA and warm-up eat the early slots' wall time
                c *= 0.5
            cap.append(c)
        cum = 0.0
        cap_cum = []
        for c in cap:
            cum += c
            cap_cum.append(cum)

        spent = 0
        for i, (tb, si) in enumerate(stream):
            # staged units whose push slot arrived enter the FIFO first
            # (before this slot's own pushes) - ordering matters for the
            # norm_b-before-next-block-PV arena constraint
            for u in [u for u in staged if u[0] <= i]:
                staged.remove(u)
                backlog.append(u[1:])
            if si == 0 and tb + 1 < TB:
                push_block_prefetch(tb + 1)
            for hp in range(2):
                if (tb, si, hp) == (0, 0, 1):
                    emit_qk(1, 0, qT, 0)
                    emit_qk(1, FC, kT, 0)
                emit_qk_si(hp, tb, si)
            # units for this slot's PV (and norm/proj at diagonal slots);
            # in the last block the chains drain inline (no next block to
            # hide them in), so their deadlines are tight
            last = tb + 1 >= TB
            if not last:
                d_pv = idx[(tb + 1, si)] - 1
            else:
                d_pv = min(i + 1, nslots - 1)
            ttl0 = max(0, si - 4 * tb)
            backlog.append((i + 1, d_pv, (4 - ttl0) * 4 * 65,
                            lambda tb=tb, si=si: emit_pv_si(tb, si)))
            if si >= 4 * tb:
                tt = si
                cell = {}

                def norm_a(tb=tb, tt=tt, cell=cell):
                    cell["y"] = emit_norm_a(tb, tt)

                def norm_b(tb=tb, tt=tt, cell=cell):
                    emit_norm_b(tb, tt, cell["y"])

                # non-last blocks: FIFO order (not deadlines) guarantees
                # norm-before-next-block-PV, so norm/proj are purely paced
                d_n = min(i + 2, nslots - 1) if last else nslots - 1
                d_p = min(i + 3, nslots - 1) if last else nslots - 1
                staged.append((i + 1, min(i + 1, nslots - 1), d_n, 0,
                               norm_a))
                staged.append((i + 1, min(i + 2, nslots - 1), d_n, 2 * 128,
                               norm_b))
                staged.append((i + 2, min(i + 3, nslots - 1), d_p, 4 * 512,
                               lambda tt=tt: emit_proj(tt)))
            # drain: everything up to the deepest due unit, then pace ready
            # units to the capacity-weighted rate
            budget = total_filler * cap_cum[i] / cap_cum[-1]
            due = max((j for j, u in enumerate(backlog) if u[1] <= i),
                      default=-1)
            while backlog and (due >= 0 or
                               (backlog[0][0] <= i and spent < budget)):
                _, _, cyc, fn = backlog.pop(0)
                due -= 1
                fn()
                spent += cyc
        for u in staged:
            backlog.append(u[1:])
        for _, _, _, fn in backlog:
            fn()


_NC_CACHE = None


def _get_nc():
    global _NC_CACHE
    if _NC_CACHE is None:
        _NC_CACHE = build()
    return _NC_CACHE


def _hilo8(a):
    hi = a.astype(NP_F8)
    lo = (a - hi.astype(np.float32)).astype(NP_F8)
    return hi, lo


def _in_maps(x, W_attn, W_proj):
    wp16 = W_proj.astype(np.float16)
    was = W_attn * WS
    maps = []
    for core in range(N_CORES):
        b, g = core // 4, core % 4
        f0 = FC * g
        xT = np.ascontiguousarray(x[b].T)           # [C, T] fp32
        xh, xl = _hilo8(xT)
        wq8 = np.ascontiguousarray(was[:, f0:f0 + FC]).astype(NP_F8)
        wk8 = np.ascontiguousarray(
            was[:, C + f0:C + f0 + FC]).astype(NP_F8)
        wv = np.ascontiguousarray(was[:, 2 * C + f0:2 * C + f0 + FC])
        wvh, wvl = _hilo8(wv)
        maps.append({
            "xhi": xh,
            "xlo": xl,
            "wqk": np.ascontiguousarray(np.concatenate([wq8, wk8], axis=1)),
            "wv2": np.ascontiguousarray(np.concatenate([wvh, wvl], axis=1)),
            "wp": np.ascontiguousarray(wp16[f0:f0 + FC, :]),
        })
    return maps


def run(x, W_attn, W_proj, trace=False, **kwargs):
    nc = _get_nc()
    res = run_bass_kernel_spmd(nc, _in_maps(x, W_attn, W_proj),
                               core_ids=list(range(N_CORES)),
                               trace=trace, **kwargs)
    out = np.zeros((B, T, C), dtype=np.float32)
    for core in range(N_CORES):
        out[core // 4] += res.results[core]["out"].astype(np.float32)
    return out, res


def kernel(x, W_attn, W_proj):
    x = np.asarray(x, dtype=np.float32)
    W_attn = np.asarray(W_attn, dtype=np.float32)
    W_proj = np.asarray(W_proj, dtype=np.float32)
    out, _ = run(x, W_attn, W_proj, trace=False)
    return out


# revision 84
# speedup vs baseline: 1.0038x; 1.0007x over previous
"""Causal self-attention Trainium2 kernel (B=2, T=2048, C=1024, H=16).

Sharding: tensor-parallel over heads (4-way) x data-parallel over batch (2-way)
= 8 cores. Core c handles batch b = c//4 and heads [4*(c%4), 4*(c%4)+4).
Each core computes x @ W_attn for its head slice, causal attention for its 4
heads, and a partial y @ W_proj over its 256 channels. The host sums the 4
partials per batch element (no device collectives).

QKV runs in fp8e4m3 DoubleRow (2 k-tiles per matmul, 0.5 cyc/row):
  - x is host-split into x_hi + x_lo (error-feedback pair, both fp8).
  - W_attn columns are host-scaled by 64 (its 0.02-scale values otherwise
    land in e4m3's subnormal range) and split hi/lo.
  - q,k use QK_PRODS products (1 = x_hi*W_hi); v uses 3 products
    (x_hi*W_hi + x_lo*W_hi + x_hi*W_lo) since v-noise feeds the output
    directly. Scale compensation: exp() scale absorbs the 64^2 on scores;
    the PV denominator ones-column is 64 so y = num/den cancels v's 64.
Scores (QK^T), PV, and the projection stay fp16 (fp8 there fails the 2e-2
gate: softmax weights and v/y are too noise-sensitive).

x arrives host-pre-transposed ([C, T] contiguous), so SBUF loads are plain
DMAs - no DMA-xbar transposes gating the start.

Layouts (per core, b fixed):
  xhi/xlo [c, t] : [128, 8 ct, 2048] fp8
  qT/kT  [d', t] : per head-pair hp, [128, 2048] fp16; partitions 0-63 =
                   head 2hp, 64-127 = head 2hp+1
  vp  [s, h, d'] : [128, 16 s-tiles, 4 heads, 65] fp16; col 64 holds 64.0
                   (v's scale) so PV emits the softmax denominator for free
  sp  [s, hi, t] : scores for one s-tile, both heads of pair hp, PSUM
                   [128,2,512]; one exp covers both heads, causally trimmed
  y2  [t, h, 65] : PV output t-major in a 3-bank PSUM arena; col 64 is the
                   denominator. Normalize is a per-partition tensor_scalar
                   multiply, then a PE transpose back to [c', t] for proj.

Emission is si-outer: per s-tile, QK -> exp -> PV matmuls for every t-tile
at or above the diagonal, so the PE has PV work during the exp-bound early
s-tiles and the per-block tail is tiny. qkv matmuls for later t-blocks are
drip-fed into the stream with per-unit deadlines. Output staging copies
run on DVE (ACT for the later tiles, where its exp stream has idle);
output is fp16, upcast and summed on host.
"""

import sys

if "/opt/trn_rl_repo" not in sys.path:
    sys.path.insert(0, "/opt/trn_rl_repo")

import numpy as np
import ml_dtypes

import concourse.bass as bass
import concourse.bacc as bacc
import concourse.mybir as mybir
import concourse.tile as tile
from concourse.bass_utils import run_bass_kernel_spmd

F32 = mybir.dt.float32
F16 = mybir.dt.float16
F8 = mybir.dt.float8e4
NP_F8 = ml_dtypes.float8_e4m3
DR = mybir.MatmulPerfMode.DoubleRow

B, T, C = 2, 2048, 1024
NH = 16              # total heads
D = 64               # head dim
N_CORES = 8
HG = 4               # heads per core
FC = HG * D          # 256 f-columns per core per q/k/v
CT = C // 128        # 8 c-tiles
TT = T // 128        # 16 t-tiles / s-tiles
TB = T // 512        # 4 t-blocks
WS = 64.0            # host-side W_attn pre-scale (e4m3 subnormal dodge)
SCALE = 1.0 / (8.0 * WS * WS)   # 1/sqrt(D) / WS^2, folded into exp()
QK_PRODS = 1         # fp8 products for q,k: 1 = xhi*Whi, 2 = +xlo*Whi


def build():
    nc = bacc.Bacc("TRN2", target_bir_lowering=False, debug=False,
                   num_devices=N_CORES)
    xhi_d = nc.dram_tensor("xhi", [C, T], F8, kind="ExternalInput").ap()
    xlo_d = nc.dram_tensor("xlo", [C, T], F8, kind="ExternalInput").ap()
    wqk_d = nc.dram_tensor("wqk", [C, 2 * FC], F8, kind="ExternalInput").ap()
    wv2_d = nc.dram_tensor("wv2", [C, 2 * FC], F8, kind="ExternalInput").ap()
    wp_d = nc.dram_tensor("wp", [FC, C], F16, kind="ExternalInput").ap()
    out_d = nc.dram_tensor("out", [T, C], F16, kind="ExternalOutput").ap()

    with tile.TileContext(nc) as tc:
        body(tc, xhi_d, xlo_d, wqk_d, wv2_d, wp_d, out_d)
    nc.compile()
    return nc


def body(tc, xhi_d, xlo_d, wqk_d, wv2_d, wp_d, out_d):
    nc = tc.nc
    Exp = mybir.ActivationFunctionType.Exp

    with (
        tc.tile_pool(name="sb", bufs=1) as sb,
        tc.tile_pool(name="ps", bufs=1, space="PSUM") as ps,
    ):
        mask = sb.tile([128, 128], F16)
        ident = sb.tile([128, 128], F16)
        ones = sb.tile([128, 64], F16)
        wqk_sb = sb.tile([128, CT, 2 * FC], F8)   # [c, ct, wq|wk]
        wv2_sb = sb.tile([128, CT, 2 * FC], F8)   # [c, ct, wvh|wvl]
        wp_sb = sb.tile([128, 2, C], F16)          # [c'(128), hp, n]
        xhi = sb.tile([128, CT, T], F8)            # [c_local, ct, t]
        xlo = sb.tile([128, CT, T], F8)
        qT = sb.tile([128, 2, T], F16)             # [d', hp, t]
        kT = sb.tile([128, 2, T], F16)
        vp = sb.tile([128, TT, HG, 65], F16)       # [s_in_tile, s_tile, h, d'+1]
        pt0 = sb.tile([128, TT, 2, 512], F16)      # [s, s_tile, hi, t_in_tb]
        pt1 = sb.tile([128, TT, 2, 512], F16)
        pts = (pt0, pt1)
        yT = sb.tile([128, 2, T], F16)             # [c', hp, t]

        # Startup critical path: each HWDGE queue (SP/ACT/DVE) serializes
        # its DMAs at ~1.3us setup + transfer, so the first-QKV inputs are
        # spread across all three queues: x chunks on SP, q/k weights on
        # the (still idle) Activation queue, v weights on the DVE queue.
        # wp (first proj, ~slot 4) comes via the Pool SWDGE queue.
        for tb in range(TB):
            sl = slice(tb * 512, (tb + 1) * 512)
            nc.sync.dma_start(
                xhi[:, :, sl],
                xhi_d[:, sl].rearrange("(ct p) t -> p ct t", p=128))
            if tb == 0:
                nc.sync.dma_start(
                    wqk_sb, wqk_d.rearrange("(ct p) f -> p ct f", p=128))
                nc.sync.dma_start(
                    wv2_sb, wv2_d.rearrange("(ct p) f -> p ct f", p=128))
            nc.sync.dma_start(
                xlo[:, :, sl],
                xlo_d[:, sl].rearrange("(ct p) t -> p ct t", p=128))

        # binary causal mask in S^T orientation: 1 where t - s >= 0 else 0
        nc.gpsimd.memset(mask, 1.0)
        nc.gpsimd.affine_select(
            out=mask, in_=mask, compare_op=mybir.AluOpType.is_ge,
            fill=0.0, base=0, pattern=[[1, 128]], channel_multiplier=-1)
        # identity for PE transposes: intersect the two triangle selects
        nc.gpsimd.memset(ident, 1.0)
        nc.gpsimd.affine_select(
            out=ident, in_=ident, compare_op=mybir.AluOpType.is_ge,
            fill=0.0, base=0, pattern=[[1, 128]], channel_multiplier=-1)
        nc.gpsimd.affine_select(
            out=ident, in_=ident, compare_op=mybir.AluOpType.is_ge,
            fill=0.0, base=0, pattern=[[-1, 128]], channel_multiplier=1)
        nc.gpsimd.memset(ones, 1.0)
        nc.gpsimd.memset(vp[:, :, :, 64:65], WS)   # PV denominator column
        nc.gpsimd.dma_start(wp_sb, wp_d.rearrange("(hp p) n -> p hp n", p=128))

        # PSUM: sp 2x2 banks + qv 1 + arena 3 = 8 banks
        def sp_tile(name):
            return ps.tile([128, 2, 512], F32, tag="sp", name=name, bufs=2)

        def qv_tile(name):
            return ps.tile([128, 512], F32, tag="qv", name=name, bufs=1)

        # 3-bank arena: 16 PV accumulation regions of 65 fp32 columns
        # (region r = ttl*4+h at bank r//7, slot r%7). PSUM start_tensor_calc
        # zeroes a whole 2KB bank, which would wipe sibling regions, so the
        # arena is zeroed by DVE memset per t-block and every PV matmul
        # accumulates with start=False.
        arena = ps.tile([128, 1536], F32, name="arena")

        def reg(h, ttl):
            r = ttl * 4 + h
            bank, idx = divmod(r, 7)
            c0 = bank * 512 + idx * 65
            return arena[:, c0:c0 + 65]

        # HAM warmup: the PE is DMA-blocked at startup; dependency-free
        # matmuls on constants un-throttle the clock, and one tiny exp
        # prefetches the spline table.
        for _ in range(6):
            warm = qv_tile("warm")
            nc.tensor.matmul(warm[0:64, 0:128], lhsT=ones, rhs=mask,
                             start=True, stop=True)
            nc.tensor.matmul(warm[0:64, 128:256], lhsT=ones, rhs=mask,
                             start=True, stop=True)
        warm_e = sb.tile([1, 64], F16)
        nc.scalar.activation(warm_e, ones[0:1, :], Exp, scale=SCALE)

        def emit_v(tt):
            # 3-product fp8 DoubleRow chain: xhi*Wh + xlo*Wh + xhi*Wl.
            # Output lives in the arena's bank-2 tail (not qv), so v chains
            # don't serialize with the q/k GEMM chains through one bank.
            # start=True would zero PV regions 14/15 in the same bank, so
            # the region is memset (Pool) and the chain runs start=False.
            v_ps = qv_tile("v_ps")[:, 0:256]
            prods = ((xhi, 0), (xlo, 0), (xhi, FC))
            n = len(prods) * 4
            i = 0
            for xp, f0 in prods:
                for j in range(4):
                    nc.tensor.matmul(
                        v_ps,
                        lhsT=xp[:, 2 * j:2 * j + 2,
                                tt * 128:(tt + 1) * 128],
                        rhs=wv2_sb[:, 2 * j:2 * j + 2, f0:f0 + FC],
                        start=(i == 0), stop=(i == n - 1), perf_mode=DR)
                    i += 1
            if tt >= 12:
                # these pop in the tb2->tb3 boundary window where the DVE
                # is the bottleneck and the exp stream has a hole
                nc.scalar.copy(
                    vp[:, tt, :, 0:64],
                    v_ps.rearrange("p (h d) -> p h d", h=HG))
            else:
                nc.vector.tensor_copy(
                    vp[:, tt, :, 0:64],
                    v_ps.rearrange("p (h d) -> p h d", h=HG))

        def emit_qk(hp, f0, dst, tb):
            qk_ps = qv_tile("qk_ps")
            xparts = (xhi, xlo)[:QK_PRODS]
            n = len(xparts) * 4
            i = 0
            for xp in xparts:
                for j in range(4):
                    nc.tensor.matmul(
                        qk_ps,
                        lhsT=wqk_sb[:, 2 * j:2 * j + 2,
                                    f0 + hp * 128:f0 + (hp + 1) * 128],
                        rhs=xp[:, 2 * j:2 * j + 2,
                               tb * 512:(tb + 1) * 512],
                        start=(i == 0), stop=(i == n - 1), perf_mode=DR)
                    i += 1
            nc.vector.tensor_copy(
                dst[:, hp, tb * 512:(tb + 1) * 512], qk_ps)

        def emit_qk_si(hp, tb, si):
            # scores S^T for one s-tile, both heads of pair hp, then one
            # exp covering both heads with the causal prefix trimmed
            kd = si - 4 * tb
            col0 = 128 * kd if kd > 0 else 0
            sp = sp_tile("sp")
            for hi in (0, 1):
                nc.tensor.matmul(
                    sp[:, hi, col0:512],
                    lhsT=kT[64 * hi:64 * hi + 64, hp,
                            si * 128:(si + 1) * 128],
                    rhs=qT[64 * hi:64 * hi + 64, hp,
                           tb * 512 + col0:(tb + 1) * 512],
                    start=True, stop=True)
            pt = pts[hp]
            nc.scalar.activation(pt[:, si, :, col0:512], sp[:, :, col0:512],
                                 Exp, scale=SCALE)
            if kd >= 0:
                # zero the invalid triangle of the diagonal square after
                # exp (exp * 0 == masked exp, off the S -> exp hot path);
                # on Pool - SBUF-only op, and the Pool queue is quiet
                for hi in (0, 1):
                    psl = pt[:, si, hi, col0:col0 + 128]
                    nc.gpsimd.tensor_mul(psl, psl, mask)

        def emit_pv_si(tb, si):
            # PV matmuls of s-tile si into every t-tile at/above the
            # diagonal; chain (h, ttl) accumulates over si and closes at
            # the diagonal
            for ttl in range(max(0, si - 4 * tb), 4):
                tt = 4 * tb + ttl
                for h in range(HG):
                    hp, hi = h // 2, h % 2
                    nc.tensor.matmul(
                        reg(h, ttl),
                        lhsT=pts[hp][:, si, hi, ttl * 128:(ttl + 1) * 128],
                        rhs=vp[:, si, h, :],
                        start=False, stop=(si == tt),
                        skip_group_check=True)

        def region_runs(ttl):
            """maximal same-bank runs of the 4 regions of t-tile ttl"""
            runs = []
            h0 = 0
            while h0 < HG:
                r0 = ttl * 4 + h0
                n = min(HG - h0, 7 - r0 % 7)
                bank, idx = divmod(r0, 7)
                runs.append((h0, n, bank * 512 + idx * 65))
                h0 += n
            return runs

        # normalize is split into two backlog units so the PE transpose is
        # emitted a slot after the DVE multiply that feeds it (the in-order
        # PE queue would otherwise stall mid-chain waiting on the DVE)
        def emit_norm_a(tb, tt):
            ttl = tt - 4 * tb
            runs = region_runs(ttl)
            rcp = sb.tile([128, HG], F32, tag="rcp", name="rcp", bufs=3)
            for h0, n, base in runs:
                sl = arena[:, base:base + 65 * n].rearrange(
                    "p (n x) -> p n x", n=n)
                nc.vector.reciprocal(rcp[:, h0:h0 + n], sl[:, :, 64])
            y_sb = sb.tile([128, FC], F16, tag="ysb", name="y_sb", bufs=3)
            for h0, n, base in runs:
                sl = arena[:, base:base + 65 * n].rearrange(
                    "p (n x) -> p n x", n=n)
                nc.vector.tensor_mul(
                    y_sb[:, h0 * 64:(h0 + n) * 64].rearrange(
                        "p (n x) -> p n x", n=n),
                    sl[:, :, 0:64],
                    rcp[:, h0:h0 + n].unsqueeze(2).broadcast_to([128, n, 64]))
            return y_sb

        def emit_norm_b(tb, tt, y_sb):
            ttl = tt - 4 * tb
            # PE transpose [t, c'] -> [c', t] for the projection lhsT; the
            # scratch borrows an sp rotation slot (start=True zeroing is
            # safe there, unlike in the accumulating arena)
            yt = qv_tile("yt")[:, 0:128].bitcast(F16)      # [128, 256] f16
            for ch in range(2):
                nc.tensor.transpose(
                    yt[:, ch * 128:(ch + 1) * 128],
                    y_sb[:, ch * 128:(ch + 1) * 128], ident)
            nc.vector.tensor_copy(
                yT[:, :, tt * 128:(tt + 1) * 128],
                yt.rearrange("p (hp t) -> p hp t", hp=2))
            # re-zero this t-tile's regions for the next block's start=False
            # accumulation chains (WAR-ordered after the reads above); the
            # last block has no successor - skipping it shortens the tail
            if tb + 1 < TB:
                for h0, n, base in region_runs(ttl):
                    nc.vector.memset(arena[:, base:base + 65 * n], 0.0)

        def emit_proj(tt):
            # two 1-bank qv passes instead of a 2-bank sp borrow: the sp
            # rotation then serves only the QK+exp stream. The last tiles
            # run after the exp stream ends, so they use the idle sp banks
            # (parallel, not qv-serialized) to shorten the tail.
            if tt >= 12:
                pjt = sp_tile("pj")
                pj = [pjt[:, 0, :], pjt[:, 1, :]]
            else:
                pj = [qv_tile("pj0"), qv_tile("pj1")]
            for nb in range(2):
                for hp in range(2):
                    nc.tensor.matmul(
                        pj[nb],
                        lhsT=yT[:, hp, tt * 128:(tt + 1) * 128],
                        rhs=wp_sb[:, hp, nb * 512:(nb + 1) * 512],
                        start=(hp == 0), stop=(hp == 1))
            # GPSIMD cannot access PSUM (BIR verifier): DVE evacuates, and
            # the Activation engine (idle once its exp stream is done)
            # takes over for the final tiles to shorten the tail
            ob = sb.tile([128, C], F16, tag="ob", name="ob", bufs=3)
            if tt >= 10:
                nc.scalar.copy(ob[:, 0:512], pj[0])
                nc.scalar.copy(ob[:, 512:1024], pj[1])
            else:
                nc.vector.tensor_copy(ob[:, 0:512], pj[0])
                nc.vector.tensor_copy(ob[:, 512:1024], pj[1])
            nc.sync.dma_start(out_d[tt * 128:(tt + 1) * 128, :], ob)

        # ---- global exp-stream emission ----
        # The Activation engine (exp) is the roofline; emission follows one
        # global (tb, si) stream so its QK+exp pairs are never queued behind
        # bulk PE work. All other PE/DVE/Pool work (qkv GEMMs, PV, norm,
        # proj) is a FIFO backlog of (ready, deadline, cycles, fn) units
        # drained between stream slots: a unit is held until its `ready`
        # slot (so cross-engine producers from slot i have a full slot of
        # wall time before an in-order consumer is emitted), forced at its
        # `deadline`, and otherwise paced to even PE-cycle rate. FIFO order
        # is load-bearing for pv->norm->proj chains and norm-before-next-
        # block-PV (arena reuse); Tile semaphores enforce the data deps.
        stream = [(tb, si) for tb in range(TB) for si in range(4 * tb + 4)]
        idx = {p: i for i, p in enumerate(stream)}
        nslots = len(stream)

        # initial zeroing of the PV regions (start=False chains)
        for ttl in range(4):
            for h0, n, base in region_runs(ttl):
                nc.vector.memset(arena[:, base:base + 65 * n], 0.0)

        # tb0 prologue: only head-pair 0's q/k GEMMs before the stream; the
        # hp1 GEMMs are emitted inside slot 0 between the hp0 and hp1
        # QK+exp pairs, so the first exp isn't queued behind them
        emit_qk(0, 0, qT, 0)
        emit_qk(0, FC, kT, 0)

        backlog = []   # (ready, deadline, pe_cycles, fn) FIFO
        staged = []    # (push_slot, ready, deadline, pe_cycles, fn)
        QKG = 256 * 4 * QK_PRODS        # emit_qk PE cycles
        VG = 128 * 12                   # emit_v PE cycles

        for tt in range(4):
            backlog.append((0, tt + 2, VG, lambda tt=tt: emit_v(tt)))

        def push_block_prefetch(tb):
            """qk GEMMs + v tiles of block tb, pushed during block tb-1."""
            i0 = idx[(tb, 0)]
            for j, (hp, f0, dst) in enumerate(
                    ((0, 0, qT), (0, FC, kT), (1, 0, qT), (1, FC, kT))):
                backlog.append((0, i0 - 3 + (j + 1) // 2, QKG,
                                lambda hp=hp, f0=f0, d=dst, tb=tb:
                                emit_qk(hp, f0, d, tb)))
            for ttl in range(4):
                tt = 4 * tb + ttl
                backlog.append((0, idx[(tb, min(tt, 4 * tb + 3))] - 1, VG,
                                lambda tt=tt: emit_v(tt)))

        total_filler = 4 * VG
        for tb in range(1, TB):
            total_filler += 4 * QKG + 4 * VG
        for tb, si in stream:
            ttl0 = max(0, si - 4 * tb)
            total_filler += (4 - ttl0) * 4 * 65          # PV
            if si >= 4 * tb:
                total_filler += 2 * 128 + 4 * 512        # norm transposes+proj

        # per-slot filler capacity = exp wall-time at 2.4GHz minus the
        # slot's mandatory QK cycles; pacing follows cumulative capacity so
        # loaded early blocks shed filler into the roomy late blocks
        cap = []
        for i, (tb, si) in enumerate(stream):
            cols = 512 - (128 * (si - 4 * tb) if si >= 4 * tb else 0)
            act_cyc = 2 * (2 * cols + 444)               # 2 exps, 1.2GHz*2
            c = max(0.0, act_cyc * 2.0 - 4 * cols)
            if i < 8:
                # startup DMA and warm-up eat the early slots' wall time
                c *= 0.5
            cap.append(c)
        cum = 0.0
        cap_cum = []
        for c in cap:
            cum += c
            cap_cum.append(cum)

        spent = 0
        for i, (tb, si) in enumerate(stream):
            # staged units whose push slot arrived enter the FIFO first
            # (before this slot's own pushes) - ordering matters for the
            # norm_b-before-next-block-PV arena constraint
            for u in [u for u in staged if u[0] <= i]:
                staged.remove(u)
                backlog.append(u[1:])
            if si == 0 and tb + 1 < TB:
                push_block_prefetch(tb + 1)
            for hp in range(2):
                if (tb, si, hp) == (0, 0, 1):
                    emit_qk(1, 0, qT, 0)
                    emit_qk(1, FC, kT, 0)
                emit_qk_si(hp, tb, si)
            # units for this slot's PV (and norm/proj at diagonal slots);
            # in the last block the chains drain inline (no next block to
            # hide them in), so their deadlines are tight
            last = tb + 1 >= TB
            if not last:
                d_pv = idx[(tb + 1, si)] - 1
            else:
                d_pv = min(i + 1, nslots - 1)
            ttl0 = max(0, si - 4 * tb)
            backlog.append((i + 1, d_pv, (4 - ttl0) * 4 * 65,
                            lambda tb=tb, si=si: emit_pv_si(tb, si)))
            if si >= 4 * tb:
                tt = si
                cell = {}

                def norm_a(tb=tb, tt=tt, cell=cell):
                    cell["y"] = emit_norm_a(tb, tt)

                def norm_b(tb=tb, tt=tt, cell=cell):
                    emit_norm_b(tb, tt, cell["y"])

                # non-last blocks: FIFO order (not deadlines) guarantees
                # norm-before-next-block-PV, so norm/proj are purely paced
                d_n = min(i + 2, nslots - 1) if last else nslots - 1
                d_p = min(i + 3, nslots - 1) if last else nslots - 1
                staged.append((i + 1, min(i + 1, nslots - 1), d_n, 0,
                               norm_a))
                staged.append((i + 1, min(i + 2, nslots - 1), d_n, 2 * 128,
                               norm_b))
                staged.append((i + 2, min(i + 3, nslots - 1), d_p, 4 * 512,
                               lambda tt=tt: emit_proj(tt)))
            # drain: everything up to the deepest due unit, then pace ready
            # units to the capacity-weighted rate
            budget = total_filler * cap_cum[i] / cap_cum[-1]
            due = max((j for j, u in enumerate(backlog) if u[1] <= i),
                      default=-1)
            while backlog and (due >= 0 or
                               (backlog[0][0] <= i and spent < budget)):
                _, _, cyc, fn = backlog.pop(0)
                due -= 1
                fn()
                spent += cyc
        for u in staged:
            backlog.append(u[1:])
        for _, _, _, fn in backlog:
            fn()


_NC_CACHE = None


def _get_nc():
    global _NC_CACHE
    if _NC_CACHE is None:
        _NC_CACHE = build()
    return _NC_CACHE


def _hilo8(a):
    hi = a.astype(NP_F8)
    lo = (a - hi.astype(np.float32)).astype(NP_F8)
    return hi, lo


def _in_maps(x, W_attn, W_proj):
    wp16 = W_proj.astype(np.float16)
    was = W_attn * WS
    maps = []
    for core in range(N_CORES):
        b, g = core // 4, core % 4
        f0 = FC * g
        xT = np.ascontiguousarray(x[b].T)           # [C, T] fp32
        xh, xl = _hilo8(xT)
        wq8 = np.ascontiguousarray(was[:, f0:f0 + FC]).astype(NP_F8)
        wk8 = np.ascontiguousarray(
            was[:, C + f0:C + f0 + FC]).astype(NP_F8)
        wv = np.ascontiguousarray(was[:, 2 * C + f0:2 * C + f0 + FC])
        wvh, wvl = _hilo8(wv)
        maps.append({
            "xhi": xh,
            "xlo": xl,
            "wqk": np.ascontiguousarray(np.concatenate([wq8, wk8], axis=1)),
            "wv2": np.ascontiguousarray(np.concatenate([wvh, wvl], axis=1)),
            "wp": np.ascontiguousarray(wp16[f0:f0 + FC, :]),
        })
    return maps


def run(x, W_attn, W_proj, trace=False, **kwargs):
    nc = _get_nc()
    res = run_bass_kernel_spmd(nc, _in_maps(x, W_attn, W_proj),
                               core_ids=list(range(N_CORES)),
                               trace=trace, **kwargs)
    out = np.zeros((B, T, C), dtype=np.float32)
    for core in range(N_CORES):
        out[core // 4] += res.results[core]["out"].astype(np.float32)
    return out, res


def kernel(x, W_attn, W_proj):
    x = np.asarray(x, dtype=np.float32)
    W_attn = np.asarray(W_attn, dtype=np.float32)
    W_proj = np.asarray(W_proj, dtype=np.float32)
    out, _ = run(x, W_attn, W_proj, trace=False)
    return out
